# revision 4
# baseline (speedup 1.0000x reference)
"""GraphSage layer on 8 trn2 NeuronCores via Bass.

Reference math (N=50000 nodes, D=128 feats, E=800000 edges):
    msgs   = edge_val[:,None] * x[edge_dst]            # [E, D]
    h1     = segment_sum(msgs, edge_src, N)            # [N, D]
    degree = segment_sum(edge_val, edge_src, N)
    h1     = h1 / (degree[:,None] + 1e-6)
    out    = concat([x, h1], 1) @ W                    # [N, 128]

Strategy (SPMD, one program for 8 cores; per-core data differs):
  - Nodes are greedily bin-packed by degree into 392 blocks of <=128 nodes
    with near-equal edge counts; core c owns blocks [49c, 49c+49).
  - Per block: edges padded to 17 tiles of 128. Each tile is one indirect
    DMA gather of x[edge_dst] rows (f32, one row per partition), then a
    one-hot matrix S[e, n] = (src_rel[e]==n) * (edge_val[e]/(deg+1e-6))
    built in one fused tensor_scalar op, then a PE matmul accumulates
    h1T[f, n-block] += msgs[e, f]^T @ S[e, n] into PSUM.
  - Output: outT[o, n] = W[:128].T @ xT_blk + W[128:].T @ h1T, written as
    [128, 6272] per core; host transposes/scatters back.
The gather (833 indirect DMA calls/core at ~1.45us each, Q7 descriptor-gen
bound) dominates; all compute overlaps under it.
"""
import sys
import types

import numpy as np

sys.path.insert(0, "/opt/trn_rl_repo")

N = 50000
D = 128
E = 800000
N_CORES = 8
BLOCKS_PER_CORE = 49
N_BLOCKS = N_CORES * BLOCKS_PER_CORE  # 392
TILES_PER_BLOCK = 17
P = 128
CALLS = BLOCKS_PER_CORE * TILES_PER_BLOCK  # 833
NODES_PER_CORE = BLOCKS_PER_CORE * P  # 6272
PAD_SRC = 999.0  # src_rel sentinel for padded edge slots -> zero one-hot col


def _ensure_axon_hooks():
    """bass_utils needs antenv.axon_hooks for trace; provide a stub if absent."""
    try:
        import antenv.axon_hooks  # noqa: F401
        return
    except ImportError:
        pass
    import antenv
    mod = types.ModuleType("antenv.axon_hooks")
    mod._hook = None

    def set_axon_ntff_profile_hook(hook):
        mod._hook = hook

    def get_axon_ntff_profile_hook():
        return mod._hook

    mod.set_axon_ntff_profile_hook = set_axon_ntff_profile_hook
    mod.get_axon_ntff_profile_hook = get_axon_ntff_profile_hook
    sys.modules["antenv.axon_hooks"] = mod
    antenv.axon_hooks = mod


def _patch_tile_drain():
    """This walrus build accepts only ONE sync-wait per instruction.
    Patch Tile lowering to split any multi-wait instruction by inserting
    single-wait nops (same engine) before it, and do the same for the
    kernel-tail drain."""
    import bass_rust
    import concourse.tile as tile_mod
    from concourse import mybir
    from concourse.vector_clock import ScopedClock

    if getattr(tile_mod.TileContext, "_wait_split_patched", False):
        return
    tile_mod.TileContext._wait_split_patched = True

    orig_commit_and_lower = tile_mod.TileContext._commit_and_lower
    counter = [0]

    def _split_commit_and_lower(self, inst, bb, old_bb_map, bb_to_exit_bb):
        si = getattr(inst, "sync_info", None)
        if si is not None and si.on_wait and len(si.on_wait) > 1:
            waits = list(si.on_wait)
            inst.sync_info = mybir.SyncInfo(
                on_wait=[waits[-1]], on_update=list(si.on_update or [])
            )
            for w in waits[:-1]:
                counter[0] += 1
                nop = bass_rust.InstNoOp(
                    name=f"waitsplit_{counter[0]}", text_hint="wait_split"
                )
                nop.engine = inst.engine
                nop.bass_nofuse = True
                nop.sync_info = mybir.SyncInfo(on_wait=[w], on_update=[])
                self._add_instruction(nop)
        orig_commit_and_lower(self, inst, bb, old_bb_map, bb_to_exit_bb)

    tile_mod.TileContext._commit_and_lower = _split_commit_and_lower

    def _patched(self, tick_clock, wait_clock):  # tail drain
        nc = self.nc
        drain_res = nc.sync.drain()
        drain_inst = drain_res.ins
        wait_clock.add_sem_waits(drain_inst, ScopedClock({None: tick_clock.global_clock}))
        si = drain_inst.sync_info
        waits = list(si.on_wait or []) if si is not None else []
        if len(waits) > 1:
            si.on_wait = waits[:1]
            bb = nc.cur_bb.bb
            nops = []
            for w in waits[1:]:
                nop_res = nc.sync.nop(nofuse=True, hint="drain_wait_split")
                nop_res.ins.sync_info = mybir.SyncInfo(on_wait=[w], on_update=[])
                nops.append(nop_res.ins)
            insts = list(bb.instructions)
            di = next(i for i, x in enumerate(insts) if x.name == drain_inst.name)
            nop_names = {n.name for n in nops}
            rest = [x for x in insts[di:] if x.name not in nop_names]
            new_order = insts[:di] + nops + rest
            try:
                bb.instructions = new_order
            except (AttributeError, TypeError):
                live = bb.instructions
                live[:] = new_order
        nc.all_engine_barrier()
        assert self.sems is not None
        popped = nc._tile_sem_poison_stack.pop()
        assert popped is self._sem_poison
        nc.clear_and_free_semaphores(list(self.sems.allocated().values()))
        nc.all_engine_barrier()

    tile_mod.TileContext._drain_and_barrier = _patched


def _partition_nodes(edge_src, edge_val):
    """Greedy balanced bin-packing of nodes into N_BLOCKS blocks.

    Returns (block_nodes [N_BLOCKS, P] int32 node ids padded with -1,
             deg [N] float64 weighted degree).
    """
    import heapq

    deg_w = np.bincount(edge_src, weights=edge_val.astype(np.float64), minlength=N)
    cnt = np.bincount(edge_src, minlength=N)
    order = np.argsort(-cnt, kind="stable")
    # heap of (edge_count, node_count, block_id)
    heap = [(0, 0, b) for b in range(N_BLOCKS)]
    heapq.heapify(heap)
    block_nodes = [[] for _ in range(N_BLOCKS)]
    pending = []
    for node in order:
        c = int(cnt[node])
        while True:
            ec, nn_, b = heapq.heappop(heap)
            if nn_ < P:
                block_nodes[b].append(int(node))
                heapq.heappush(heap, (ec + c, nn_ + 1, b))
                for it in pending:
                    heapq.heappush(heap, it)
                pending = []
                break
            pending.append((ec, nn_, b))
    out = np.full((N_BLOCKS, P), -1, dtype=np.int64)
    for b in range(N_BLOCKS):
        ns = block_nodes[b]
        out[b, :len(ns)] = ns
    return out, deg_w


def _build_core_data(x, W, edge_src, edge_dst, edge_val):
    """Host-side sharding: returns per-core input dicts + scatter map."""
    edge_src = np.asarray(edge_src)
    edge_dst = np.asarray(edge_dst)
    edge_val = np.asarray(edge_val, dtype=np.float32)
    x = np.asarray(x, dtype=np.float32)
    W = np.asarray(W, dtype=np.float32)

    block_nodes, deg_w = _partition_nodes(edge_src, edge_val)

    # node -> (block, row)
    node_block = np.full(N, -1, dtype=np.int32)
    node_row = np.full(N, -1, dtype=np.int32)
    for b in range(N_BLOCKS):
        ns = block_nodes[b]
        valid = ns >= 0
        node_block[ns[valid]] = b
        node_row[ns[valid]] = np.nonzero(valid)[0]

    # group edges by block of their src
    eb = node_block[edge_src]
    order = np.argsort(eb, kind="stable")
    es_sorted = edge_src[order]
    ed_sorted = edge_dst[order]
    ev_sorted = edge_val[order]
    block_edge_counts = np.bincount(eb, minlength=N_BLOCKS)
    max_edges = int(block_edge_counts.max())
    assert max_edges <= TILES_PER_BLOCK * P, (
        f"block overflow: {max_edges} > {TILES_PER_BLOCK * P}"
    )
    block_edge_starts = np.zeros(N_BLOCKS + 1, dtype=np.int64)
    np.cumsum(block_edge_counts, out=block_edge_starts[1:])

    scale_per_edge = (ev_sorted / (deg_w[es_sorted] + 1e-6)).astype(np.float32)

    in_maps = []
    for c in range(N_CORES):
        idx_arr = np.zeros((P, CALLS), dtype=np.int32)
        srcrel_arr = np.full((P, CALLS), PAD_SRC, dtype=np.float32)
        sc_arr = np.zeros((P, CALLS), dtype=np.float32)
        xT = np.zeros((D, NODES_PER_CORE), dtype=np.float32)
        for bi in range(BLOCKS_PER_CORE):
            b = c * BLOCKS_PER_CORE + bi
            s, e = block_edge_starts[b], block_edge_starts[b + 1]
            k = e - s
            dsts = ed_sorted[s:e]
            # sort the block's edges by dst for DRAM read locality
            o2 = np.argsort(dsts, kind="stable")
            dsts = dsts[o2]
            rows = node_row[es_sorted[s:e]][o2].astype(np.float32)
            scs = scale_per_edge[s:e][o2]
            # slot j*128+p  ->  call (bi*17 + j), partition p
            jj = np.arange(k) // P + bi * TILES_PER_BLOCK
            pp = np.arange(k) % P
            idx_arr[pp, jj] = dsts
            srcrel_arr[pp, jj] = rows
            sc_arr[pp, jj] = scs
            ns = block_nodes[b]
            valid = ns >= 0
            xT[:, bi * P:bi * P + int(valid.sum())] = x[ns[valid]].T
        in_maps.append({
            "x_table": x,
            "xT": xT,
            "W": W,
            "idx": idx_arr,
            "srcrel": srcrel_arr,
            "sc": sc_arr,
        })
    return in_maps, block_nodes


def _build_program():
    from concourse import bass, mybir
    import concourse.tile as tile

    nc = bass.Bass()
    dt = mybir.dt.float32
    x_table = nc.declare_dram_parameter("x_table", [N, D], dt, isOutput=False)
    xT = nc.declare_dram_parameter("xT", [D, NODES_PER_CORE], dt, isOutput=False)
    Wp = nc.declare_dram_parameter("W", [2 * D, D], dt, isOutput=False)
    idx = nc.declare_dram_parameter("idx", [P, CALLS], mybir.dt.int32, isOutput=False)
    srcrel = nc.declare_dram_parameter("srcrel", [P, CALLS], dt, isOutput=False)
    sc = nc.declare_dram_parameter("sc", [P, CALLS], dt, isOutput=False)
    outT = nc.declare_dram_parameter("outT", [D, NODES_PER_CORE], dt, isOutput=True)

    with tile.TileContext(nc) as tc:
        with (
            tc.tile_pool(name="const", bufs=1) as cpool,
            tc.tile_pool(name="msgs", bufs=6) as mpool,
            tc.tile_pool(name="st", bufs=6) as stpool,
            tc.tile_pool(name="sb", bufs=3) as sbpool,
            tc.tile_pool(name="psum", bufs=2, space="PSUM") as pspool,
            tc.tile_pool(name="psum_out", bufs=2, space="PSUM") as pspool2,
        ):
            idx_t = cpool.tile([P, CALLS], mybir.dt.int32)
            srcrel_t = cpool.tile([P, CALLS], dt)
            sc_t = cpool.tile([P, CALLS], dt)
            xT_t = cpool.tile([D, NODES_PER_CORE], dt)
            w1_t = cpool.tile([D, D], dt)
            w2_t = cpool.tile([D, D], dt)
            iota_t = cpool.tile([P, P], dt)
            nc.sync.dma_start(out=idx_t[:], in_=idx[:])
            nc.sync.dma_start(out=srcrel_t[:], in_=srcrel[:])
            nc.sync.dma_start(out=sc_t[:], in_=sc[:])
            nc.sync.dma_start(out=xT_t[:], in_=xT[:])
            nc.sync.dma_start(out=w1_t[:], in_=Wp[0:D, :])
            nc.sync.dma_start(out=w2_t[:], in_=Wp[D:2 * D, :])
            nc.gpsimd.iota(iota_t[:], pattern=[[1, P]], base=0,
                           channel_multiplier=0,
                           allow_small_or_imprecise_dtypes=True)

            for bi in range(BLOCKS_PER_CORE):
                h1_ps = pspool.tile([D, P], mybir.dt.float32, tag="h1")
                for j in range(TILES_PER_BLOCK):
                    col = bi * TILES_PER_BLOCK + j
                    msgs = mpool.tile([P, D], dt, tag="msgs")
                    nc.gpsimd.indirect_dma_start(
                        out=msgs[:], out_offset=None, in_=x_table[:],
                        in_offset=bass.IndirectOffsetOnAxis(
                            ap=idx_t[:, col:col + 1], axis=0),
                    )
                    st = stpool.tile([P, P], dt, tag="st")
                    nc.vector.tensor_scalar(
                        out=st[:], in0=iota_t[:],
                        scalar1=srcrel_t[:, col:col + 1],
                        scalar2=sc_t[:, col:col + 1],
                        op0=mybir.AluOpType.is_equal,
                        op1=mybir.AluOpType.mult,
                    )
                    nc.tensor.matmul(
                        out=h1_ps[:], lhsT=msgs[:], rhs=st[:],
                        start=(j == 0), stop=(j == TILES_PER_BLOCK - 1),
                    )
                h1_sb = sbpool.tile([D, P], dt, tag="h1sb")
                nc.vector.tensor_copy(out=h1_sb[:], in_=h1_ps[:])
                out_ps = pspool2.tile([D, P], mybir.dt.float32, tag="outp")
                nc.tensor.matmul(out=out_ps[:], lhsT=w1_t[:],
                                 rhs=xT_t[:, bi * P:(bi + 1) * P],
                                 start=True, stop=False)
                nc.tensor.matmul(out=out_ps[:], lhsT=w2_t[:], rhs=h1_sb[:],
                                 start=False, stop=True)
                out_sb = sbpool.tile([D, P], dt, tag="outsb")
                nc.vector.tensor_copy(out=out_sb[:], in_=out_ps[:])
                nc.sync.dma_start(out=outT[:, bi * P:(bi + 1) * P], in_=out_sb[:])
    return nc


def kernel(x, W, edge_src, edge_dst, edge_val):
    _ensure_axon_hooks()
    _patch_tile_drain()
    from concourse.bass_utils import run_bass_kernel_spmd

    in_maps, block_nodes = _build_core_data(x, W, edge_src, edge_dst, edge_val)
    nc = _build_program()
    res = run_bass_kernel_spmd(nc, in_maps, list(range(N_CORES)))
    out = np.zeros((N, D), dtype=np.float32)
    for c in range(N_CORES):
        oT = res.results[c]["outT"]  # [D, NODES_PER_CORE]
        for bi in range(BLOCKS_PER_CORE):
            b = c * BLOCKS_PER_CORE + bi
            ns = block_nodes[b]
            valid = ns >= 0
            out[ns[valid]] = oT[:, bi * P:bi * P + int(valid.sum())].T
    return out


# revision 7
# speedup vs baseline: 1.0813x; 1.0813x over previous
"""GraphSage layer on 8 trn2 NeuronCores via Bass.

Reference math (N=50000 nodes, D=128 feats, E=800000 edges):
    msgs   = edge_val[:,None] * x[edge_dst]            # [E, D]
    h1     = segment_sum(msgs, edge_src, N)            # [N, D]
    degree = segment_sum(edge_val, edge_src, N)
    h1     = h1 / (degree[:,None] + 1e-6)
    out    = concat([x, h1], 1) @ W                    # [N, 128]

Strategy (SPMD, one program for 8 cores; per-core data differs):
  - Nodes are greedily bin-packed by degree into 392 blocks of <=128 nodes
    with near-equal edge counts; core c owns blocks [49c, 49c+49).
  - Per block: edges padded to 17 tiles of 128. Each tile is one indirect
    DMA gather of x[edge_dst] rows (f32, one row per partition), then a
    one-hot matrix S[e, n] = (src_rel[e]==n) * (edge_val[e]/(deg+1e-6))
    built in one fused tensor_scalar op, then a PE matmul accumulates
    h1T[f, n-block] += msgs[e, f]^T @ S[e, n] into PSUM.
  - Output: outT[o, n] = W[:128].T @ xT_blk + W[128:].T @ h1T, written as
    [128, 6272] per core; host transposes/scatters back.
The gather (833 indirect DMA calls/core at ~1.45us each, Q7 descriptor-gen
bound) dominates; all compute overlaps under it.
"""
import sys
import types

import numpy as np

sys.path.insert(0, "/opt/trn_rl_repo")

N = 50000
D = 128
E = 800000
N_CORES = 8
BLOCKS_PER_CORE = 49
N_BLOCKS = N_CORES * BLOCKS_PER_CORE  # 392
P = 128
NODES_PER_CORE = BLOCKS_PER_CORE * P  # 6272
PAD_SRC = 999.0  # src_rel sentinel for padded edge slots -> zero one-hot col


def _ensure_axon_hooks():
    """bass_utils needs antenv.axon_hooks for trace; provide a stub if absent."""
    try:
        import antenv.axon_hooks  # noqa: F401
        return
    except ImportError:
        pass
    import antenv
    mod = types.ModuleType("antenv.axon_hooks")
    mod._hook = None

    def set_axon_ntff_profile_hook(hook):
        mod._hook = hook

    def get_axon_ntff_profile_hook():
        return mod._hook

    mod.set_axon_ntff_profile_hook = set_axon_ntff_profile_hook
    mod.get_axon_ntff_profile_hook = get_axon_ntff_profile_hook
    sys.modules["antenv.axon_hooks"] = mod
    antenv.axon_hooks = mod


def _patch_tile_drain():
    """This walrus build accepts only ONE sync-wait per instruction.
    Patch Tile lowering to split any multi-wait instruction by inserting
    single-wait nops (same engine) before it, and do the same for the
    kernel-tail drain."""
    import bass_rust
    import concourse.tile as tile_mod
    from concourse import mybir
    from concourse.vector_clock import ScopedClock

    if getattr(tile_mod.TileContext, "_wait_split_patched", False):
        return
    tile_mod.TileContext._wait_split_patched = True

    orig_commit_and_lower = tile_mod.TileContext._commit_and_lower
    counter = [0]

    def _split_commit_and_lower(self, inst, bb, old_bb_map, bb_to_exit_bb):
        si = getattr(inst, "sync_info", None)
        if si is not None and si.on_wait and len(si.on_wait) > 1:
            waits = list(si.on_wait)
            inst.sync_info = mybir.SyncInfo(
                on_wait=[waits[-1]], on_update=list(si.on_update or [])
            )
            for w in waits[:-1]:
                counter[0] += 1
                nop = bass_rust.InstNoOp(
                    name=f"waitsplit_{counter[0]}", text_hint="wait_split"
                )
                nop.engine = inst.engine
                nop.bass_nofuse = True
                nop.sync_info = mybir.SyncInfo(on_wait=[w], on_update=[])
                self._add_instruction(nop)
        orig_commit_and_lower(self, inst, bb, old_bb_map, bb_to_exit_bb)

    tile_mod.TileContext._commit_and_lower = _split_commit_and_lower

    def _patched(self, tick_clock, wait_clock):  # tail drain
        nc = self.nc
        drain_res = nc.sync.drain()
        drain_inst = drain_res.ins
        wait_clock.add_sem_waits(drain_inst, ScopedClock({None: tick_clock.global_clock}))
        si = drain_inst.sync_info
        waits = list(si.on_wait or []) if si is not None else []
        if len(waits) > 1:
            si.on_wait = waits[:1]
            bb = nc.cur_bb.bb
            nops = []
            for w in waits[1:]:
                nop_res = nc.sync.nop(nofuse=True, hint="drain_wait_split")
                nop_res.ins.sync_info = mybir.SyncInfo(on_wait=[w], on_update=[])
                nops.append(nop_res.ins)
            insts = list(bb.instructions)
            di = next(i for i, x in enumerate(insts) if x.name == drain_inst.name)
            nop_names = {n.name for n in nops}
            rest = [x for x in insts[di:] if x.name not in nop_names]
            new_order = insts[:di] + nops + rest
            try:
                bb.instructions = new_order
            except (AttributeError, TypeError):
                live = bb.instructions
                live[:] = new_order
        nc.all_engine_barrier()
        assert self.sems is not None
        popped = nc._tile_sem_poison_stack.pop()
        assert popped is self._sem_poison
        nc.clear_and_free_semaphores(list(self.sems.allocated().values()))
        nc.all_engine_barrier()

    tile_mod.TileContext._drain_and_barrier = _patched


def _partition_nodes(edge_src, edge_val):
    """Greedy balanced bin-packing of nodes into N_BLOCKS blocks.

    Returns (block_nodes [N_BLOCKS, P] int32 node ids padded with -1,
             deg [N] float64 weighted degree).
    """
    import heapq

    deg_w = np.bincount(edge_src, weights=edge_val.astype(np.float64), minlength=N)
    cnt = np.bincount(edge_src, minlength=N)
    order = np.argsort(-cnt, kind="stable")
    # heap of (edge_count, node_count, block_id)
    heap = [(0, 0, b) for b in range(N_BLOCKS)]
    heapq.heapify(heap)
    block_nodes = [[] for _ in range(N_BLOCKS)]
    pending = []
    for node in order:
        c = int(cnt[node])
        while True:
            ec, nn_, b = heapq.heappop(heap)
            if nn_ < P:
                block_nodes[b].append(int(node))
                heapq.heappush(heap, (ec + c, nn_ + 1, b))
                for it in pending:
                    heapq.heappush(heap, it)
                pending = []
                break
            pending.append((ec, nn_, b))
    out = np.full((N_BLOCKS, P), -1, dtype=np.int64)
    for b in range(N_BLOCKS):
        ns = block_nodes[b]
        out[b, :len(ns)] = ns
    return out, deg_w


def _build_core_data(x, W, edge_src, edge_dst, edge_val):
    """Host-side sharding: returns per-core input dicts + scatter map."""
    edge_src = np.asarray(edge_src)
    edge_dst = np.asarray(edge_dst)
    edge_val = np.asarray(edge_val, dtype=np.float32)
    x = np.asarray(x, dtype=np.float32)
    W = np.asarray(W, dtype=np.float32)

    block_nodes, deg_w = _partition_nodes(edge_src, edge_val)

    # node -> (block, row)
    node_block = np.full(N, -1, dtype=np.int32)
    node_row = np.full(N, -1, dtype=np.int32)
    for b in range(N_BLOCKS):
        ns = block_nodes[b]
        valid = ns >= 0
        node_block[ns[valid]] = b
        node_row[ns[valid]] = np.nonzero(valid)[0]

    # group edges by block of their src
    eb = node_block[edge_src]
    order = np.argsort(eb, kind="stable")
    es_sorted = edge_src[order]
    ed_sorted = edge_dst[order]
    ev_sorted = edge_val[order]
    block_edge_counts = np.bincount(eb, minlength=N_BLOCKS)
    max_edges = int(block_edge_counts.max())
    tiles_per_block = -(-max_edges // P)  # ceil; edge-balanced pack -> 16
    block_edge_starts = np.zeros(N_BLOCKS + 1, dtype=np.int64)
    np.cumsum(block_edge_counts, out=block_edge_starts[1:])

    scale_per_edge = (ev_sorted / (deg_w[es_sorted] + 1e-6)).astype(np.float32)

    calls = BLOCKS_PER_CORE * tiles_per_block
    in_maps = []
    for c in range(N_CORES):
        idx_arr = np.zeros((P, calls), dtype=np.int32)
        srcrel_arr = np.full((P, calls), PAD_SRC, dtype=np.float32)
        sc_arr = np.zeros((P, calls), dtype=np.float32)
        xT = np.zeros((D, NODES_PER_CORE), dtype=np.float32)
        for bi in range(BLOCKS_PER_CORE):
            b = c * BLOCKS_PER_CORE + bi
            s, e = block_edge_starts[b], block_edge_starts[b + 1]
            k = e - s
            dsts = ed_sorted[s:e]
            # sort the block's edges by dst for DRAM read locality
            o2 = np.argsort(dsts, kind="stable")
            dsts = dsts[o2]
            rows = node_row[es_sorted[s:e]][o2].astype(np.float32)
            scs = scale_per_edge[s:e][o2]
            # slot j*128+p  ->  call (bi*17 + j), partition p
            jj = np.arange(k) // P + bi * tiles_per_block
            pp = np.arange(k) % P
            idx_arr[pp, jj] = dsts
            srcrel_arr[pp, jj] = rows
            sc_arr[pp, jj] = scs
            ns = block_nodes[b]
            valid = ns >= 0
            xT[:, bi * P:bi * P + int(valid.sum())] = x[ns[valid]].T
        in_maps.append({
            "x_table": x,
            "xT": xT,
            "W": W,
            "idx": idx_arr,
            "srcrel": srcrel_arr,
            "sc": sc_arr,
        })
    return in_maps, block_nodes, tiles_per_block


def _build_program(tiles_per_block):
    from concourse import bass, mybir
    import concourse.tile as tile

    nc = bass.Bass()
    calls = BLOCKS_PER_CORE * tiles_per_block
    dt = mybir.dt.float32
    x_table = nc.declare_dram_parameter("x_table", [N, D], dt, isOutput=False)
    xT = nc.declare_dram_parameter("xT", [D, NODES_PER_CORE], dt, isOutput=False)
    Wp = nc.declare_dram_parameter("W", [2 * D, D], dt, isOutput=False)
    idx = nc.declare_dram_parameter("idx", [P, calls], mybir.dt.int32, isOutput=False)
    srcrel = nc.declare_dram_parameter("srcrel", [P, calls], dt, isOutput=False)
    sc = nc.declare_dram_parameter("sc", [P, calls], dt, isOutput=False)
    outT = nc.declare_dram_parameter("outT", [D, NODES_PER_CORE], dt, isOutput=True)

    with tile.TileContext(nc) as tc:
        with (
            tc.tile_pool(name="const", bufs=1) as cpool,
            tc.tile_pool(name="msgs", bufs=6) as mpool,
            tc.tile_pool(name="st", bufs=6) as stpool,
            tc.tile_pool(name="sb", bufs=3) as sbpool,
            tc.tile_pool(name="psum", bufs=2, space="PSUM") as pspool,
            tc.tile_pool(name="psum_out", bufs=2, space="PSUM") as pspool2,
        ):
            idx_t = cpool.tile([P, calls], mybir.dt.int32)
            srcrel_t = cpool.tile([P, calls], dt)
            sc_t = cpool.tile([P, calls], dt)
            xT_t = cpool.tile([D, NODES_PER_CORE], dt)
            w1_t = cpool.tile([D, D], dt)
            w2_t = cpool.tile([D, D], dt)
            iota_t = cpool.tile([P, P], dt)
            nc.sync.dma_start(out=idx_t[:], in_=idx[:])
            nc.sync.dma_start(out=srcrel_t[:], in_=srcrel[:])
            nc.sync.dma_start(out=sc_t[:], in_=sc[:])
            nc.sync.dma_start(out=xT_t[:], in_=xT[:])
            nc.sync.dma_start(out=w1_t[:], in_=Wp[0:D, :])
            nc.sync.dma_start(out=w2_t[:], in_=Wp[D:2 * D, :])
            nc.gpsimd.iota(iota_t[:], pattern=[[1, P]], base=0,
                           channel_multiplier=0,
                           allow_small_or_imprecise_dtypes=True)

            for bi in range(BLOCKS_PER_CORE):
                h1_ps = pspool.tile([D, P], mybir.dt.float32, tag="h1")
                for j in range(tiles_per_block):
                    col = bi * tiles_per_block + j
                    msgs = mpool.tile([P, D], dt, tag="msgs")
                    nc.gpsimd.indirect_dma_start(
                        out=msgs[:], out_offset=None, in_=x_table[:],
                        in_offset=bass.IndirectOffsetOnAxis(
                            ap=idx_t[:, col:col + 1], axis=0),
                    )
                    st = stpool.tile([P, P], dt, tag="st")
                    nc.vector.tensor_scalar(
                        out=st[:], in0=iota_t[:],
                        scalar1=srcrel_t[:, col:col + 1],
                        scalar2=sc_t[:, col:col + 1],
                        op0=mybir.AluOpType.is_equal,
                        op1=mybir.AluOpType.mult,
                    )
                    nc.tensor.matmul(
                        out=h1_ps[:], lhsT=msgs[:], rhs=st[:],
                        start=(j == 0), stop=(j == tiles_per_block - 1),
                    )
                h1_sb = sbpool.tile([D, P], dt, tag="h1sb")
                nc.vector.tensor_copy(out=h1_sb[:], in_=h1_ps[:])
                out_ps = pspool2.tile([D, P], mybir.dt.float32, tag="outp")
                nc.tensor.matmul(out=out_ps[:], lhsT=w1_t[:],
                                 rhs=xT_t[:, bi * P:(bi + 1) * P],
                                 start=True, stop=False)
                nc.tensor.matmul(out=out_ps[:], lhsT=w2_t[:], rhs=h1_sb[:],
                                 start=False, stop=True)
                out_sb = sbpool.tile([D, P], dt, tag="outsb")
                nc.vector.tensor_copy(out=out_sb[:], in_=out_ps[:])
                nc.sync.dma_start(out=outT[:, bi * P:(bi + 1) * P], in_=out_sb[:])
    return nc


def kernel(x, W, edge_src, edge_dst, edge_val):
    _ensure_axon_hooks()
    _patch_tile_drain()
    from concourse.bass_utils import run_bass_kernel_spmd

    in_maps, block_nodes, tiles_per_block = _build_core_data(
        x, W, edge_src, edge_dst, edge_val)
    nc = _build_program(tiles_per_block)
    res = run_bass_kernel_spmd(nc, in_maps, list(range(N_CORES)))
    out = np.zeros((N, D), dtype=np.float32)
    for c in range(N_CORES):
        oT = res.results[c]["outT"]  # [D, NODES_PER_CORE]
        for bi in range(BLOCKS_PER_CORE):
            b = c * BLOCKS_PER_CORE + bi
            ns = block_nodes[b]
            valid = ns >= 0
            out[ns[valid]] = oT[:, bi * P:bi * P + int(valid.sum())].T
    return out
